# revision 1
# baseline (speedup 1.0000x reference)
"""CapsNet forward pass on 8 Trainium2 NeuronCores (pure data parallelism).

Pipeline per core (4 images):
  conv1 (256x1x30x160, stride (1,4)) as 40 accumulated PE matmuls per 8-row
  block using a phase-decomposed Toeplitz layout (K=(s,i)=120 partitions,
  width shifts via free-dim AP offsets) in bf16,
  -> primary caps conv (256x256x10x10, stride 10 = non-overlapping) as 200
  accumulated PE matmuls with the conv1 output as the stationary operand so
  the result lands transposed [spatial, channel] (routing-friendly),
  -> squash + per-capsule 8x80 matmuls on the vector engine
  (per-partition-scalar MACs), -> 4 rounds of agreement routing
  (DVE/ACT elementwise + ones-matmul partition reduction),
  -> class head + log_softmax on device. Host only shards/stacks.
"""

import numpy as np
import ml_dtypes
from contextlib import ExitStack

import concourse.bass as bass
import concourse.tile as tile
import concourse.mybir as mybir
from concourse.bass_utils import run_bass_kernel_spmd

F32 = mybir.dt.float32
BF16 = mybir.dt.bfloat16
AF = mybir.ActivationFunctionType
ALU = mybir.AluOpType

# Problem constants
N_CORES = 8
IMG = 4            # images per core
HI, WI = 120, 640  # input image
KH, KW = 30, 160   # conv1 kernel
S = 4              # conv1 width stride (phase count)
A = KW // S        # 40 width taps per phase
HO, WO = 91, 121   # conv1 output
C1 = 256
HB = 4             # conv1 row-block (4*121=484 fp32 <= one PSUM bank)
PQ = 100           # prim kernel positions (10x10)
EF = 108           # prim output spatial (9*12)
D2 = 256           # prim output channels
G = 32             # capsule groups (i-tiles)
NK = 8             # capsule input dim
OD = 80            # 5 classes * 16
NCL, FD = 5, 16
NITER = 4          # initial softmax round + 3 routing iterations
XFLAT = HO * WO    # 11011


def _emit(nc):
    """Emit the whole per-core program into nc (inside a TileContext)."""
    import os
    stage = os.environ.get("K_STAGE", "all")  # conv|prim|caps|all (timing bisect)
    # ---- DRAM I/O ----
    d_inp = nc.dram_tensor("inp", [IMG, HI, WI], F32, kind="ExternalInput")
    d_w1t = nc.dram_tensor("w1t", [A, S * KH, C1], BF16, kind="ExternalInput")
    d_b1 = nc.dram_tensor("b1", [128, 2], F32, kind="ExternalInput")
    d_wpt = nc.dram_tensor("wpt", [PQ, C1, D2], BF16, kind="ExternalInput")
    d_bp = nc.dram_tensor("bp", [D2], F32, kind="ExternalInput")
    d_cw = nc.dram_tensor("cw", [G * EF, NK * OD], F32, kind="ExternalInput")
    d_br = nc.dram_tensor("br", [G * EF, NCL], F32, kind="ExternalInput")
    d_wc = nc.dram_tensor("wc", [26, FD], F32, kind="ExternalInput")
    d_bc = nc.dram_tensor("bc", [26, 1], F32, kind="ExternalInput")
    d_out = nc.dram_tensor("out", [26, IMG * NCL], F32, kind="ExternalOutput")

    # ---- persistent SBUF tensors (raw; deps still tracked by Tile) ----
    t_w1t = nc.alloc_sbuf_tensor("s_w1t", [S * KH, A, C1], BF16)
    t_b1 = nc.alloc_sbuf_tensor("s_b1", [128, 2], F32)
    t_bp = nc.alloc_sbuf_tensor("s_bp", [1, D2], F32)
    t_bpx = nc.alloc_sbuf_tensor("s_bpx", [EF, D2], F32)
    t_onesr = nc.alloc_sbuf_tensor("s_onesr", [1, EF], F32)
    t_wc = nc.alloc_sbuf_tensor("s_wc", [FD + 1, 26], F32)
    t_ones = nc.alloc_sbuf_tensor("s_ones", [EF, 1], F32)
    t_raw = [nc.alloc_sbuf_tensor(f"s_raw{i}", [HI, WI], F32) for i in range(2)]
    t_P = [nc.alloc_sbuf_tensor(f"s_P{i}", [HI, S, KW], BF16) for i in range(2)]
    t_yd = [nc.alloc_sbuf_tensor(f"s_yd{i}", [S * KH, HB, KW], BF16) for i in range(3)]
    t_x = [nc.alloc_sbuf_tensor(f"s_x{c}", [128, 10800], BF16) for c in range(2)]
    t_u = nc.alloc_sbuf_tensor("s_u", [EF, G, IMG, NK], F32)
    t_up = nc.alloc_sbuf_tensor("s_up", [EF, G, IMG, OD], F32)
    t_sq = nc.alloc_sbuf_tensor("s_sq", [EF, G, NK], F32)
    t_l2 = nc.alloc_sbuf_tensor("s_l2", [EF, IMG, G], F32)
    t_f1 = nc.alloc_sbuf_tensor("s_f1", [EF, IMG, G], F32)
    t_f2 = nc.alloc_sbuf_tensor("s_f2", [EF, IMG, G], F32)
    t_f3 = nc.alloc_sbuf_tensor("s_f3", [EF, IMG, G], F32)
    t_bb = nc.alloc_sbuf_tensor("s_bb", [EF, G, IMG, NCL], F32)
    t_ce = nc.alloc_sbuf_tensor("s_ce", [EF, G, IMG, NCL], F32)
    t_cc = nc.alloc_sbuf_tensor("s_cc", [EF, G, IMG, NCL], F32)
    t_cs = nc.alloc_sbuf_tensor("s_cs", [EF, G * IMG], F32)
    t_cr = nc.alloc_sbuf_tensor("s_cr", [EF, G * IMG], F32)
    t_tg = nc.alloc_sbuf_tensor("s_tg", [EF, IMG, NCL, FD], F32)
    t_bt = nc.alloc_sbuf_tensor("s_bt", [EF, IMG, NCL], F32)
    t_sa = nc.alloc_sbuf_tensor("s_sa", [EF, IMG * OD], F32)
    t_v = nc.alloc_sbuf_tensor("s_v", [1, IMG * OD], F32)
    t_sv = nc.alloc_sbuf_tensor("s_sv", [1, IMG * OD], F32)
    t_l2v = nc.alloc_sbuf_tensor("s_l2v", [1, IMG * NCL], F32)
    t_f1v = nc.alloc_sbuf_tensor("s_f1v", [1, IMG * NCL], F32)
    t_f2v = nc.alloc_sbuf_tensor("s_f2v", [1, IMG * NCL], F32)
    t_f3v = nc.alloc_sbuf_tensor("s_f3v", [1, IMG * NCL], F32)
    t_wc2 = nc.alloc_sbuf_tensor("s_wc2", [26, FD], F32)
    t_bc = nc.alloc_sbuf_tensor("s_bc", [26, 1], F32)
    t_hm = nc.alloc_sbuf_tensor("s_hm", [26, IMG * OD], F32)
    t_li = nc.alloc_sbuf_tensor("s_li", [26, IMG * NCL], F32)
    t_ee = nc.alloc_sbuf_tensor("s_ee", [26, IMG * NCL], F32)
    t_ss = nc.alloc_sbuf_tensor("s_ss", [1, IMG * NCL], F32)
    t_ln = nc.alloc_sbuf_tensor("s_ln", [1, IMG * NCL], F32)
    t_lg = nc.alloc_sbuf_tensor("s_lg", [26, IMG * NCL], F32)
    t_jk = nc.alloc_sbuf_tensor("s_jk", [1, 8], F32)
    t_brs = nc.alloc_sbuf_tensor("s_brs", [EF, G * NCL], F32)

    with tile.TileContext(nc) as tc, ExitStack() as ctx:
        wp_pool = ctx.enter_context(tc.tile_pool(name="wp", bufs=24))
        cw_pool = ctx.enter_context(tc.tile_pool(name="cwp", bufs=3))
        cps = ctx.enter_context(tc.tile_pool(name="cps", bufs=2, space="PSUM"))
        pps = ctx.enter_context(tc.tile_pool(name="pps", bufs=1, space="PSUM"))
        sps = ctx.enter_context(tc.tile_pool(name="sps", bufs=1, space="PSUM"))

        # ---- load constants ----
        nc.sync.dma_start(
            out=t_w1t.ap(),
            in_=bass.AP(tensor=d_w1t, offset=0,
                        ap=[[C1, S * KH], [S * KH * C1, A], [1, C1]]),
        )
        nc.sync.dma_start(out=t_b1.ap(), in_=d_b1.ap())
        nc.sync.dma_start(
            out=t_bpx.ap(), in_=bass.AP(tensor=d_bp, offset=0, ap=[[0, EF], [1, D2]])
        )
        nc.sync.dma_start(out=t_wc2.ap(), in_=d_wc.ap())
        nc.sync.dma_start(out=t_bc.ap(), in_=d_bc.ap())
        nc.vector.memset(t_ones.ap(), 1.0)
        nc.vector.memset(t_onesr.ap(), 1.0)
        nc.vector.memset(t_lg.ap(), 0.0)
        # ACT sync stubs: absorb foreign-proc waits one at a time so no ACT
        # instruction ever needs >1 hardware sync-wait slot (ACT descriptors
        # only encode a single wait).
        nc.scalar.copy(out=t_jk.ap()[0:1, 0:1], in_=t_b1.ap()[0:1, 0:1])
        nc.scalar.copy(out=t_jk.ap()[0:1, 1:2], in_=t_ones.ap()[0:1, 0:1])
        c0ap = nc.const_aps.scalar_like(0.0, t_jk.ap()[0:1, 0:1])
        nc.scalar.copy(out=t_jk.ap()[0:1, 2:3], in_=c0ap[0:1, 0:1])
        # b_route -> bb: DMA to staging, then all-DVE copies (keeps the
        # routing softmax's producers on a single engine sem)
        nc.sync.dma_start(
            out=t_brs.ap(),
            in_=bass.AP(tensor=d_br, offset=0,
                        ap=[[NCL, EF], [EF * NCL, G], [1, NCL]]),
        )
        brs3 = t_brs.ap().rearrange("p (g o) -> p g o", g=G)
        for im in range(IMG):
            nc.vector.tensor_copy(out=t_bb.ap()[:, :, im, :], in_=brs3)

        u3 = t_u.ap()
        up3 = t_up.ap()
        bp3 = t_bpx.ap().rearrange("p (g k) -> p g k", g=G)

        # ================= per-image: conv1 -> prim -> squash -> caps =====
        for im in range(IMG):
            raw = t_raw[im % 2]
            P = t_P[im % 2]
            nc.sync.dma_start(out=raw.ap(), in_=d_inp.ap()[im])
            # de-interleave width phases: P[r, s, m] = raw[r, 4m+s], cast bf16
            nc.vector.tensor_copy(
                out=P.ap(), in_=raw.ap().rearrange("p (m s) -> p s m", s=S)
            )

            # ---- conv1 ----
            for bi, h0 in enumerate(range(0, HO, HB)):
                hb = min(HB, HO - h0)
                yd = t_yd[bi % 3]
                ydap_w = yd.ap()
                engs = [nc.sync, nc.gpsimd]
                for d in range(hb):
                    # yd[(s,i), d, m] = P[h0+d+i, s, m]
                    for s_ in range(S):
                        eng = engs[(d * S + s_) % 2]
                        eng.dma_start(
                            out=ydap_w[s_ * KH:(s_ + 1) * KH, d, :],
                            in_=P.ap()[h0 + d:h0 + d + KH, s_, :],
                        )
                ps = [cps.tile([128, HB, WO], F32, tag=f"c{c2}", name=f"ps{c2}")
                      for c2 in range(2)]
                ydap = yd.ap()
                w1ap = t_w1t.ap()
                for a in range(A):
                    rhs = ydap[:, :hb, a:a + WO]
                    for c2 in range(2):
                        nc.tensor.matmul(
                            ps[c2][:, :hb, :],
                            w1ap[:, a, c2 * 128:(c2 + 1) * 128],
                            rhs,
                            start=(a == 0),
                            stop=(a == A - 1),
                        )
                for c2 in range(2):
                    for d in range(hb):
                        h = h0 + d
                        if h >= 90:
                            continue  # row 90 unused by the stride-10 prim conv
                        p_, e_ = h % 10, h // 10
                        nc.scalar.activation(
                            out=bass.AP(tensor=t_x[c2], offset=p_ * 1080 + e_ * 12,
                                        ap=[[10800, 128], [108, 10], [1, 12]]),
                            in_=ps[c2][:, d, 0:120].rearrange(
                                "p (f q) -> p q f", q=10),
                            func=AF.Relu,
                            bias=t_b1.ap()[:, c2:c2 + 1],
                            scale=1.0,
                        )

            # ---- primary caps conv (output transposed: [spatial, channel]) ----
            if stage == "conv":
                continue
            pp = pps.tile([EF, D2], F32, tag="pp", name="pp")
            for pq in range(PQ):
                wp = wp_pool.tile([128, 2, D2], BF16, name="wp")
                for c2 in range(2):
                    weng = nc.scalar if c2 == 0 else nc.gpsimd
                    weng.dma_start(
                        out=wp[:, c2, :],
                        in_=d_wpt.ap()[pq, c2 * 128:(c2 + 1) * 128, :],
                    )
                for c2 in range(2):
                    lhsT = t_x[c2].ap()[:, pq * EF:(pq + 1) * EF]
                    nc.tensor.matmul(
                        pp[:],
                        lhsT,
                        wp[:, c2, :],
                        start=(pq == 0 and c2 == 0),
                        stop=(pq == PQ - 1 and c2 == 1),
                    )
            # evac + prim bias -> u (pre-squash), u[ef, g, im, k]
            nc.vector.tensor_tensor(
                out=u3[:, :, im, :],
                in0=pp[:].rearrange("p (g k) -> p g k", g=G),
                in1=bp3,
                op=ALU.add,
            )

            # ---- squash along k ----
            if stage == "prim":
                continue
            nc.vector.tensor_tensor(
                out=t_sq.ap(), in0=u3[:, :, im, :], in1=u3[:, :, im, :], op=ALU.mult
            )
            nc.vector.tensor_reduce(
                out=t_l2.ap()[:, im, :], in_=t_sq.ap(), axis=mybir.AxisListType.X,
                op=ALU.add,
            )
            nc.scalar.sqrt(t_f1.ap()[:, im, :], t_l2.ap()[:, im, :])
            nc.scalar.copy(out=t_jk.ap()[0:1, 6:7], in_=t_f1.ap()[0:1, im, 0:1])
            nc.vector.tensor_scalar_add(t_f2.ap()[:, im, :], t_l2.ap()[:, im, :], 1.0)
            nc.vector.reciprocal(t_f3.ap()[:, im, :], t_f2.ap()[:, im, :])
            nc.vector.tensor_tensor(
                out=t_f1.ap()[:, im, :], in0=t_f1.ap()[:, im, :],
                in1=t_f3.ap()[:, im, :], op=ALU.mult,
            )
            fac = t_f1.ap()[:, im, :].unsqueeze(2).broadcast_to([EF, G, NK])
            # note: t_f1 layout is [EF, IMG, G]; u slice is [EF, G, NK] -> need g
            # aligned: t_f1[:, im, :] is [EF, G] with g innermost. OK.
            nc.vector.tensor_tensor(
                out=u3[:, :, im, :], in0=u3[:, :, im, :], in1=fac, op=ALU.mult
            )

            # ---- capsule prediction: up[ef, g, im, (o,d)] ----
            for g in range(G):
                cwt = cw_pool.tile([EF, NK, OD], F32, name="cwt")
                nc.scalar.dma_start(
                    out=cwt[:],
                    in_=d_cw.ap()[g * EF:(g + 1) * EF, :].rearrange(
                        "p (c j) -> p c j", c=NK),
                )
                for c in range(NK):
                    uscal = u3[:, g:g + 1, im:im + 1, c:c + 1]
                    if c == 0:
                        nc.vector.tensor_scalar(
                            out=up3[:, g, im, :], in0=cwt[:, c, :],
                            scalar1=uscal, scalar2=None, op0=ALU.mult,
                        )
                    else:
                        nc.vector.scalar_tensor_tensor(
                            out=up3[:, g, im, :], in0=cwt[:, c, :], scalar=uscal,
                            in1=up3[:, g, im, :], op0=ALU.mult, op1=ALU.add,
                        )

        # ================= agreement routing =================
        bb3 = t_bb.ap()
        if stage in ("conv", "prim", "caps"):
            nc.sync.dma_start(out=d_out.ap(), in_=t_lg.ap()[0:26, :])
            return
        for it in range(NITER):
            if it > 0:
                # bb += sum_d up * v; broadcast v across partitions via K=1 matmul
                vx = sps.tile([EF, IMG * OD], F32, tag="rt", name="vx")
                nc.tensor.matmul(vx[:], t_onesr.ap(), t_v.ap(),
                                 start=True, stop=True)
                vbc = vx[:].rearrange("p (i o d) -> p i o d", o=NCL, d=FD)
                for g in range(G):
                    nc.vector.tensor_tensor(
                        out=t_tg.ap(),
                        in0=up3[:, g, :, :].rearrange("p i (o d) -> p i o d", o=NCL),
                        in1=vbc,
                        op=ALU.mult,
                    )
                    nc.vector.tensor_reduce(
                        out=t_bt.ap(), in_=t_tg.ap(), axis=mybir.AxisListType.X,
                        op=ALU.add,
                    )
                    nc.vector.tensor_tensor(
                        out=bb3[:, g, :, :], in0=bb3[:, g, :, :], in1=t_bt.ap(),
                        op=ALU.add,
                    )
            # c = softmax(bb) over classes
            nc.scalar.activation(
                out=t_ce.ap().rearrange("p g i o -> p (g i o)"),
                in_=bb3.rearrange("p g i o -> p (g i o)"),
                func=AF.Exp,
            )
            nc.scalar.copy(out=t_jk.ap()[0:1, 4:5],
                           in_=t_ce.ap()[0:1, 0:1, 0, 0:1])
            nc.vector.tensor_reduce(
                out=t_cs.ap(),
                in_=t_ce.ap().rearrange("p g i o -> p (g i) o"),
                axis=mybir.AxisListType.X,
                op=ALU.add,
            )
            nc.vector.reciprocal(t_cr.ap(), t_cs.ap())
            nc.vector.tensor_tensor(
                out=t_cc.ap().rearrange("p g i o -> p (g i) o"),
                in0=t_ce.ap().rearrange("p g i o -> p (g i) o"),
                in1=t_cr.ap().unsqueeze(2).broadcast_to([EF, G * IMG, NCL]),
                op=ALU.mult,
            )
            # s = sum_i c * up  (accumulate over g, then partition-reduce)
            sacc = t_sa.ap()
            for g in range(G):
                cbc = t_cc.ap()[:, g, :, :].unsqueeze(3).broadcast_to(
                    [EF, IMG, NCL, FD])
                if g == 0:
                    nc.vector.tensor_tensor(
                        out=sacc.rearrange("p (i o d) -> p i o d", o=NCL, d=FD),
                        in0=up3[:, g, :, :].rearrange(
                            "p i (o d) -> p i o d", o=NCL),
                        in1=cbc, op=ALU.mult,
                    )
                else:
                    nc.vector.tensor_tensor(
                        out=t_tg.ap(),
                        in0=up3[:, g, :, :].rearrange(
                            "p i (o d) -> p i o d", o=NCL),
                        in1=cbc, op=ALU.mult,
                    )
                    nc.vector.tensor_tensor(
                        out=sacc, in0=sacc,
                        in1=t_tg.ap().rearrange("p i o d -> p (i o d)"),
                        op=ALU.add,
                    )
            sp = sps.tile([1, IMG * OD], F32, tag="rt", name="sp")
            nc.tensor.matmul(sp[:], t_ones.ap(), sacc, start=True, stop=True)
            # v = squash(s) on partition 0
            nc.vector.tensor_copy(out=t_sv.ap(), in_=sp[:])
            nc.vector.tensor_tensor(
                out=t_v.ap(), in0=t_sv.ap(), in1=t_sv.ap(), op=ALU.mult
            )
            nc.vector.tensor_reduce(
                out=t_l2v.ap(),
                in_=t_v.ap().rearrange("p (io d) -> p io d", d=FD),
                axis=mybir.AxisListType.X, op=ALU.add,
            )
            nc.scalar.sqrt(t_f1v.ap(), t_l2v.ap())
            nc.scalar.copy(out=t_jk.ap()[0:1, 5:6], in_=t_f1v.ap()[0:1, 0:1])
            nc.vector.tensor_scalar_add(t_f2v.ap(), t_l2v.ap(), 1.0)
            nc.vector.reciprocal(t_f3v.ap(), t_f2v.ap())
            nc.vector.tensor_tensor(
                out=t_f1v.ap(), in0=t_f1v.ap(), in1=t_f3v.ap(), op=ALU.mult
            )
            nc.vector.tensor_tensor(
                out=t_v.ap().rearrange("p (io d) -> p io d", d=FD),
                in0=t_sv.ap().rearrange("p (io d) -> p io d", d=FD),
                in1=t_f1v.ap().unsqueeze(2).broadcast_to([1, IMG * NCL, FD]),
                op=ALU.mult,
            )

        # ================= class head + log_softmax =================
        # transpose-free: broadcast v onto 26 partitions via K=1 ones-matmul,
        # contract d on the vector engine, softmax-over-partitions via a
        # second ones-matmul; final tensor stays [26 logits, 20 rows] and the
        # host transposes.
        vh = sps.tile([26, IMG * OD], F32, tag="hd", name="vh")
        nc.tensor.matmul(vh[:], t_onesr.ap()[0:1, 0:26], t_v.ap(),
                         start=True, stop=True)
        nc.vector.tensor_tensor(
            out=t_hm.ap().rearrange("p (io d) -> p io d", d=FD),
            in0=vh[:].rearrange("p (io d) -> p io d", d=FD),
            in1=t_wc2.ap().unsqueeze(1).broadcast_to([26, IMG * NCL, FD]),
            op=ALU.mult,
        )
        nc.vector.tensor_reduce(
            out=t_li.ap(),
            in_=t_hm.ap().rearrange("p (io d) -> p io d", d=FD),
            axis=mybir.AxisListType.X, op=ALU.add,
        )
        nc.vector.tensor_scalar(
            out=t_li.ap(), in0=t_li.ap(), scalar1=t_bc.ap(), scalar2=None,
            op0=ALU.add,
        )
        nc.scalar.activation(out=t_ee.ap(), in_=t_li.ap(), func=AF.Exp)
        es = sps.tile([1, IMG * NCL], F32, tag="hd", name="es")
        nc.tensor.matmul(es[:], t_ones.ap()[0:26, 0:1], t_ee.ap(),
                         start=True, stop=True)
        nc.vector.tensor_copy(out=t_ss.ap(), in_=es[:])
        nc.scalar.activation(out=t_ln.ap(), in_=t_ss.ap(), func=AF.Ln)
        lb = sps.tile([26, IMG * NCL], F32, tag="hd", name="lb")
        nc.tensor.matmul(lb[:], t_onesr.ap()[0:1, 0:26], t_ln.ap(),
                         start=True, stop=True)
        nc.vector.tensor_tensor(out=t_lg.ap(), in0=t_li.ap(), in1=lb[:],
                                op=ALU.subtract)
        nc.sync.dma_start(out=d_out.ap(), in_=t_lg.ap())

    return nc


def _legalize_waits(nc, max_waits=1):
    """Split multi-sem waits into single-wait NOP prefixes on the same engine.

    walrus's CoreV2 codegen encodes at most one sync-wait command per
    instruction descriptor; the Tile scheduler freely emits several. An
    engine-local NOP chain that performs the extra waits first is
    semantics-preserving (the engine stalls in program order either way).
    """
    n = 0
    for f in nc.m.functions:
        for b in f.blocks:
            il = b.instructions
            out = []
            for inst in il:
                si = inst.sync_info
                if si is not None and si.on_wait and len(si.on_wait) > max_waits:
                    waits = list(si.on_wait)
                    for w in waits[:-max_waits]:
                        n += 1
                        nop = mybir.InstNoOp(
                            name=f"I-waitfix-{n}", ins=[], outs=[],
                            engine=inst.engine,
                            sync_info=mybir.SyncInfo(on_wait=[w], on_update=[]),
                        )
                        nc.inst_map[nop.name] = nop
                        out.append(nop)
                    inst.sync_info = mybir.SyncInfo(
                        on_wait=waits[-max_waits:], on_update=list(si.on_update)
                    )
                out.append(inst)
            il[:] = out
    return n


_CACHE = {}


def build_nc():
    if "nc" not in _CACHE:
        nc = bass.Bass("TRN2", target_bir_lowering=False, debug=False)
        _emit(nc)
        _legalize_waits(nc)
        _CACHE["nc"] = nc
    return _CACHE["nc"]


def host_prep(inputs):
    """Preprocess weights on host into device layouts (shared by all cores)."""
    w1 = np.asarray(inputs["conv1_w"], np.float32)      # (256,1,30,160)
    w1t = (w1[:, 0].reshape(C1, KH, A, S).transpose(2, 3, 1, 0)
           .reshape(A, S * KH, C1).astype(ml_dtypes.bfloat16))
    wp = np.asarray(inputs["prim_w"], np.float32)       # (256,256,10,10)
    wpt = (wp.transpose(2, 3, 1, 0).reshape(PQ, C1, D2)
           .astype(ml_dtypes.bfloat16))
    cw = np.asarray(inputs["caps_w"], np.float32).reshape(G * EF, NK * OD)
    br = np.asarray(inputs["b_route"], np.float32)
    pred_w = np.asarray(inputs["pred_w"], np.float32)
    eos_w = np.asarray(inputs["eos_w"], np.float32)
    pred_b = np.asarray(inputs["pred_b"], np.float32)
    eos_b = np.asarray(inputs["eos_b"], np.float32)
    wc = np.concatenate([pred_w, eos_w], 0)             # (26,16)
    bc = np.concatenate([pred_b, eos_b], 0)[:, None]    # (26,1)
    return {
        "w1t": np.ascontiguousarray(w1t),
        "b1": np.ascontiguousarray(
            np.asarray(inputs["conv1_b"], np.float32).reshape(2, 128).T),
        "wpt": np.ascontiguousarray(wpt),
        "bp": np.asarray(inputs["prim_b"], np.float32),
        "cw": np.ascontiguousarray(cw),
        "br": np.ascontiguousarray(br),
        "wc": np.ascontiguousarray(wc),
        "bc": np.ascontiguousarray(bc),
    }


def make_in_maps(inputs):
    shared = host_prep(inputs)
    x = np.asarray(inputs["input"], np.float32)  # (32,1,120,640)
    in_maps = []
    for c in range(N_CORES):
        m = dict(shared)
        m["inp"] = np.ascontiguousarray(x[c * IMG:(c + 1) * IMG, 0])
        in_maps.append(m)
    return in_maps


def _get_runner():
    """Build (once) a jitted shard_map over the bass_exec custom call, so
    repeated kernel() invocations skip JAX retracing/compilation."""
    if "runner" in _CACHE:
        return _CACHE["runner"]
    import jax
    import concourse.mybir as _mybir
    from jax.sharding import Mesh, PartitionSpec
    from jax.experimental.shard_map import shard_map
    from concourse import bass2jax
    bass2jax.install_neuronx_cc_hook()
    nc = build_nc()
    pname = nc.partition_id_tensor.name if nc.partition_id_tensor else None
    in_names, out_names, out_avals, zero_outs = [], [], [], []
    for alloc in nc.m.functions[0].allocations:
        if not isinstance(alloc, _mybir.MemoryLocationSet):
            continue
        name = alloc.memorylocations[0].name
        if alloc.kind == "ExternalInput":
            if name != pname:
                in_names.append(name)
        elif alloc.kind == "ExternalOutput":
            out_names.append(name)
            shape = tuple(alloc.tensor_shape)
            dtype = _mybir.dt.np(alloc.dtype)
            out_avals.append(jax.core.ShapedArray(shape, dtype))
            zero_outs.append(np.zeros(shape, dtype))
    n_params = len(in_names)
    all_names = in_names + out_names
    if pname is not None:
        all_names = all_names + [pname]

    def _body(*args):
        operands = list(args)
        if pname is not None:
            operands.append(bass2jax.partition_id_tensor())
        outs = bass2jax._bass_exec_p.bind(
            *operands,
            out_avals=tuple(out_avals),
            in_names=tuple(all_names),
            out_names=tuple(out_names),
            lowering_input_output_aliases=(),
            sim_require_finite=True,
            sim_require_nnan=True,
            nc=nc,
        )
        return tuple(outs)

    devices = jax.devices()[:N_CORES]
    mesh = Mesh(np.asarray(devices), ("core",))
    n_outs = len(out_names)
    sharded = jax.jit(
        shard_map(_body, mesh=mesh,
                  in_specs=(PartitionSpec("core"),) * (n_params + n_outs),
                  out_specs=(PartitionSpec("core"),) * n_outs,
                  check_rep=False),
        donate_argnums=tuple(range(n_params, n_params + n_outs)),
        keep_unused=True,
    )
    _CACHE["runner"] = (sharded, in_names, out_names, zero_outs)
    return _CACHE["runner"]


def run_sharded(in_maps):
    sharded, in_names, out_names, zero_outs = _get_runner()
    concat_in = [
        np.concatenate([np.asarray(in_maps[c][n]) for c in range(N_CORES)], axis=0)
        for n in in_names
    ]
    concat_zero = [np.concatenate([z] * N_CORES, axis=0) for z in zero_outs]
    outs = sharded(*concat_in, *concat_zero)
    res = []
    for c in range(N_CORES):
        m = {}
        for i, n in enumerate(out_names):
            arr = np.asarray(outs[i])
            per = arr.shape[0] // N_CORES
            m[n] = arr[c * per:(c + 1) * per]
        res.append(m)
    return res


def kernel(**inputs):
    in_maps = make_in_maps(inputs)
    res = run_sharded(in_maps)
    out = np.concatenate(
        [np.asarray(r["out"], np.float32).T.reshape(IMG, NCL, 26) for r in res],
        axis=0,
    )
    return out

